# revision 1
# baseline (speedup 1.0000x reference)
"""Multi-head self-attention (B=4, N=1024, D=1024, H=16) on 8 Trainium2 NeuronCores.

Sharding: core c handles batch b = c//2 and head-half hh = c%2 (8 of 16 heads).
Each core computes Q/K/V projections for its (batch, head-half), the full
attention for its 8 heads, and a partial output projection over its 512
head-dims.  The host sums the two partial outputs per batch (pairwise
reduce) and adds the output bias.

Device algorithm (all matmuls bf16 inputs, f32 PSUM accumulation):
  QT[dh, n]  = sum_e WqT[e, dh] * xT[e, n]      (+ bq per-partition bias add)
  KT[dh, n]  likewise
  V[n, dh]   = sum_e xT[e, n] * WvT[e, dh]      (+ bv via rank-1 matmul)
  eT[k, q]   = sum_d KT[d, k] * QT[d, q]        per head, K=64 row-paired
  PT[k, q]   = exp(eT * DK^-0.5 + maskbias[k])  (mask -> -30000 -> exp==0)
  attnT'[m,q]= sum_k V'[k, m] * PT[k, q]        V' has a ones column -> row 64
                                                 of attnT' is the softmax sum s
  at[dh, n]  = attnT'[dh, n] / s[n]             1/s via PE broadcast of s then
                                                 full-width DVE reciprocal
  y[n, e]    = sum_dh at[dh, n] * WoT[dh, e]    partial over this core's dh

Schedule: energy/exp for head pairs is interleaved with the Q/K/V
projections so the ScalarE exp stream (the 2nd-busiest engine) starts
early and overlaps PE work for the whole kernel.  Input DMAs are spread
over two queues (sync + gpsimd).  y is DMA'd directly from PSUM.
"""
import os
import sys
import time

for _p in (
    "/opt/trn_rl_repo",
    "/root/.axon_site",
    "/root/.axon_site/_ro/trn_rl_repo",
    "/root/.axon_site/_ro/pypackages",
):
    if os.path.isdir(_p) and _p not in sys.path:
        sys.path.append(_p)

import numpy as np
import ml_dtypes

import concourse.bacc as bacc
import concourse.tile as tile
from concourse import mybir
from concourse.bass_utils import run_bass_kernel_spmd

B, N, D, H = 4, 1024, 1024, 16
DK = D // H          # 64
NCORES = 8
HPC = H // 2         # 8 heads per core
DPC = D // 2         # 512 head-dims per core
NT = N // 128        # 8 token tiles
ET = D // 128        # 8 model-dim tiles
DT = DPC // 128      # 4 head-dim tiles (one per head pair)
SCALE = float(DK) ** -0.5
MASK_NEG = -30000.0
F32 = mybir.dt.float32
BF16 = mybir.dt.bfloat16

_CACHE = {}


def _build():
    nc = bacc.Bacc("TRN2", target_bir_lowering=False, debug=False,
                   num_devices=NCORES)
    xT = nc.dram_tensor("xT", [D, N], BF16, kind="ExternalInput")
    wq = nc.dram_tensor("wq", [D, DPC], BF16, kind="ExternalInput")
    wk = nc.dram_tensor("wk", [D, DPC], BF16, kind="ExternalInput")
    wv = nc.dram_tensor("wv", [D, DPC], BF16, kind="ExternalInput")
    wo = nc.dram_tensor("wo", [DPC, D], BF16, kind="ExternalInput")
    bq = nc.dram_tensor("bq", [128, DT], F32, kind="ExternalInput")
    bk = nc.dram_tensor("bk", [128, DT], F32, kind="ExternalInput")
    bv = nc.dram_tensor("bv", [1, DPC], BF16, kind="ExternalInput")
    mb = nc.dram_tensor("mb", [128, NT], F32, kind="ExternalInput")
    y01 = nc.dram_tensor("y01_part", [N, D], F32, kind="ExternalOutput")
    y23 = nc.dram_tensor("y23_part", [N, D], F32, kind="ExternalOutput")

    with tile.TileContext(nc) as tc:
        with tc.tile_pool(name="sb", bufs=1) as sb, \
             tc.tile_pool(name="work", bufs=2) as wp, \
             tc.tile_pool(name="ps", bufs=2, space="PSUM") as ps:

            # ---------------- persistent SBUF + input loads ----------------
            # queue A (sync): wq + xT, needed first for the Q projection.
            # queue B (gpsimd): everything else.
            xT_sb = sb.tile([128, ET, N], BF16)
            wq_sb = sb.tile([128, ET, DPC], BF16)
            wk_sb = sb.tile([128, ET, DPC], BF16)
            wv_sb = sb.tile([128, ET, DPC], BF16)
            wo_sb = sb.tile([128, DT, D], BF16)
            bq_sb = sb.tile([128, DT], F32)
            bk_sb = sb.tile([128, DT], F32)
            mb_sb = sb.tile([128, NT], F32)
            bv_sb = sb.tile([1, DPC], BF16)

            nc.gpsimd.dma_start(out=xT_sb[:, 0, :], in_=xT.ap()[0:128, :])
            nc.gpsimd.dma_start(out=bq_sb, in_=bq.ap())
            nc.gpsimd.dma_start(out=bk_sb, in_=bk.ap())
            nc.gpsimd.dma_start(out=mb_sb, in_=mb.ap())
            nc.gpsimd.dma_start(out=bv_sb, in_=bv.ap())
            for et in range(0, ET):
                nc.sync.dma_start(out=wq_sb[:, et, :],
                                  in_=wq.ap()[et * 128:(et + 1) * 128, :])
                if et > 0:
                    nc.sync.dma_start(out=xT_sb[:, et, :],
                                      in_=xT.ap()[et * 128:(et + 1) * 128, :])
                nc.gpsimd.dma_start(out=wk_sb[:, et, :],
                                    in_=wk.ap()[et * 128:(et + 1) * 128, :])
                nc.gpsimd.dma_start(out=wv_sb[:, et, :],
                                    in_=wv.ap()[et * 128:(et + 1) * 128, :])
            for dt in range(DT):
                nc.gpsimd.dma_start(out=wo_sb[:, dt, :],
                                    in_=wo.ap()[dt * 128:(dt + 1) * 128, :])

            ones128 = sb.tile([1, 128], BF16)
            nc.vector.memset(ones128, 1.0)
            ones64 = sb.tile([1, 64], BF16)
            nc.vector.memset(ones64, 1.0)

            qt_sb = sb.tile([128, DT, N], BF16)
            kt_sb = sb.tile([128, DT, N], BF16)
            v_sb = sb.tile([128, NT, HPC, DK + 1], BF16)
            at_sb = sb.tile([128, DT, N], BF16)

            # ---------------- Q/K projections ----------------
            # et-outer over a dt pair: PE starts as soon as the first
            # 128-row slice of x/W arrives instead of waiting for the
            # whole tensor.
            def proj_qk2(dt0, w_sb, b_sb, dst):
                pq = [ps.tile([128, N], F32, tag="mm", name=f"pqk{dt0+i}")
                      for i in range(2)]
                for et in range(ET):
                    for i in range(2):
                        dt = dt0 + i
                        for half in range(2):
                            qs = slice(half * 512, (half + 1) * 512)
                            nc.tensor.matmul(pq[i][:, qs],
                                             w_sb[:, et, dt * 128:(dt + 1) * 128],
                                             xT_sb[:, et, qs],
                                             start=(et == 0),
                                             stop=(et == ET - 1))
                for i in range(2):
                    dt = dt0 + i
                    nc.vector.tensor_scalar_add(dst[:, dt, :], pq[i],
                                                b_sb[:, dt:dt + 1])

            # ---------------- V projection ----------------
            def proj_v(nt):
                pv = ps.tile([128, 512], F32, tag="att", name=f"pv{nt}")
                for et in range(ET):
                    nc.tensor.matmul(pv, xT_sb[:, et, nt * 128:(nt + 1) * 128],
                                     wv_sb[:, et, :],
                                     start=(et == 0), stop=False)
                nc.tensor.matmul(pv, ones128, bv_sb, start=False, stop=True)
                nc.vector.tensor_copy(
                    out=v_sb[:, nt, :, 0:DK],
                    in_=pv.rearrange("p (h d) -> p h d", h=HPC))
                nc.vector.memset(v_sb[:, nt, :, DK:DK + 1], 1.0)

            # ---------------- attention: energies + exp ----------------
            pt = {}

            def attn_e_kt(p, kt):
                eA = ps.tile([128, N], F32, tag="mm", name=f"eA{p}_{kt}")
                eB = ps.tile([128, N], F32, tag="mm", name=f"eB{p}_{kt}")
                ptA, ptB = pt[p]
                for half in range(2):
                    qs = slice(half * 512, (half + 1) * 512)
                    ks = slice(kt * 128, (kt + 1) * 128)
                    nc.tensor.matmul(eA[:, qs], kt_sb[0:64, p, ks],
                                     qt_sb[0:64, p, qs],
                                     start=True, stop=True)
                    nc.tensor.matmul(eB[:, qs], kt_sb[64:128, p, ks],
                                     qt_sb[64:128, p, qs],
                                     start=True, stop=True)
                nc.scalar.activation(ptA[:, kt, :], eA,
                                     mybir.ActivationFunctionType.Exp,
                                     bias=mb_sb[:, kt:kt + 1], scale=SCALE)
                nc.scalar.activation(ptB[:, kt, :], eB,
                                     mybir.ActivationFunctionType.Exp,
                                     bias=mb_sb[:, kt:kt + 1], scale=SCALE)

            # ---------------- attention: P @ V' ----------------
            av = {}

            def attn_av_kt(p, kt, halves=(0, 1)):
                aA, aB = av[p]
                ptA, ptB = pt[p]
                for half in halves:
                    qs = slice(half * 512, (half + 1) * 512)
                    nc.tensor.matmul(aA[:, qs], v_sb[:, kt, 2 * p, :],
                                     ptA[:, kt, qs],
                                     start=(kt == 0), stop=(kt == NT - 1))
                    nc.tensor.matmul(aB[:, qs], v_sb[:, kt, 2 * p + 1, :],
                                     ptB[:, kt, qs],
                                     start=(kt == 0), stop=(kt == NT - 1))

            def pt_alloc(p):
                pt[p] = (wp.tile([128, NT, N], BF16, tag="pt", bufs=4,
                                 name=f"ptA{p}"),
                         wp.tile([128, NT, N], BF16, tag="pt", bufs=4,
                                 name=f"ptB{p}"))

            def av_alloc(p):
                av[p] = (ps.tile([65, N], F32, tag="att", name=f"aA{p}"),
                         ps.tile([65, N], F32, tag="att", name=f"aB{p}"))

            # -------- softmax normalization (1/s broadcast via PE) --------
            # processed per q-half so downstream y-projection tiles that
            # touch only the first 512 tokens can start before the second
            # half of the chain finishes
            fin_t = {}

            def attn_fin(p, halves=(0, 1)):
                aA, aB = av[p]
                if 0 in halves:
                    fin_t[p] = (
                        wp.tile([1, N], BF16, tag="sA", name=f"sA_{p}"),
                        wp.tile([1, N], BF16, tag="sB", name=f"sB_{p}"),
                        ps.tile([128, N], F32, tag="mm", name=f"srep{p}"),
                        wp.tile([128, N], BF16, tag="srep", name=f"srsb{p}"),
                    )
                sA, sB, srep_ps, srep_sb = fin_t[p]
                w = slice(halves[0] * 512, (halves[-1] + 1) * 512)
                nc.vector.tensor_copy(out=sA[:, w], in_=aA[64:65, w])
                if p >= 2:
                    # ACT is exp-free here; shortens the critical DVE chain
                    nc.scalar.copy(sB[:, w], aB[64:65, w])
                else:
                    # keep the ACT exp stream clean mid-kernel
                    nc.vector.tensor_copy(out=sB[:, w], in_=aB[64:65, w])
                for half in halves:
                    qs = slice(half * 512, (half + 1) * 512)
                    nc.tensor.matmul(srep_ps[0:64, qs], ones64, sA[:, qs],
                                     start=True, stop=True)
                    nc.tensor.matmul(srep_ps[64:128, qs], ones64, sB[:, qs],
                                     start=True, stop=True,
                                     tile_position=(0, 64))
                with nc.allow_low_precision(reason="softmax 1/s in bf16"):
                    nc.vector.reciprocal(srep_sb[:, w], srep_ps[:, w])
                nc.vector.tensor_tensor(out=at_sb[0:64, p, w],
                                        in0=aA[0:64, w],
                                        in1=srep_sb[0:64, w],
                                        op=mybir.AluOpType.mult)
                nc.vector.tensor_tensor(out=at_sb[64:128, p, w],
                                        in0=aB[0:64, w],
                                        in1=srep_sb[64:128, w],
                                        op=mybir.AluOpType.mult)

            # ---------------- output projection (two partials) ------------
            # y01 (head pairs 0-1) runs as soon as fin(1) is done, filling
            # the PE gap while pair 3 finishes; y23 is the tail.  The two
            # partials go to DRAM separately and the host sums them.
            def yprojp(nt, dts, ydram, copy_eng):
                yp = ps.tile([128, N], F32, tag="mm", name=f"yp{dts[0]}_{nt}")
                ns = slice(nt * 128, (nt + 1) * 128)
                for half in range(2):
                    qs = slice(half * 512, (half + 1) * 512)
                    for dt in dts:
                        nc.tensor.matmul(yp[:, qs], at_sb[:, dt, ns],
                                         wo_sb[:, dt, qs],
                                         start=(dt == dts[0]),
                                         stop=(dt == dts[-1]))
                ysb = wp.tile([128, N], F32, tag="y", bufs=4,
                              name=f"ysb{dts[0]}_{nt}")
                if copy_eng == "act":
                    nc.scalar.copy(ysb, yp)
                elif copy_eng == "split":
                    nc.scalar.copy(ysb[:, 0:512], yp[:, 0:512])
                    nc.vector.tensor_copy(out=ysb[:, 512:1024],
                                          in_=yp[:, 512:1024])
                else:
                    nc.vector.tensor_copy(out=ysb, in_=yp)
                nc.sync.dma_start(out=ydram.ap()[ns, 0:512], in_=ysb[:, 0:512])
                nc.gpsimd.dma_start(out=ydram.ap()[ns, 512:1024],
                                    in_=ysb[:, 512:1024])

            # ------------- emission order (software pipeline) -------------
            # The PE instruction queue is strictly in-order, so energy
            # matmuls (which gate on ScalarE exp draining their PSUM
            # tiles) are interleaved kt-by-kt with blocks that use only
            # the "att" PSUM slots (V projection, P@V') or only "mm"
            # when no energy block is active (y projection).
            proj_qk2(0, wq_sb, bq_sb, qt_sb)
            proj_qk2(0, wk_sb, bk_sb, kt_sb)
            pt_alloc(0)
            for kt in range(NT):
                attn_e_kt(0, kt)
                if kt < 6:
                    proj_v(kt)
            pt_alloc(1)
            # V6/V7 emitted (and their att-tag tiles allocated) BEFORE
            # av_alloc(0) grabs both att slots, else deadlock
            attn_e_kt(1, 0)
            proj_v(6)
            attn_e_kt(1, 1)
            proj_v(7)
            av_alloc(0)
            attn_av_kt(0, 0)
            attn_av_kt(0, 1)
            for kt in range(2, NT):
                attn_e_kt(1, kt)
                attn_av_kt(0, kt)
            proj_qk2(2, wq_sb, bq_sb, qt_sb)
            attn_fin(0)
            proj_qk2(2, wk_sb, bk_sb, kt_sb)
            pt_alloc(2)
            av_alloc(1)
            for kt in range(NT):
                attn_e_kt(2, kt)
                attn_av_kt(1, kt)
            attn_fin(1)
            pt_alloc(3)
            av_alloc(2)
            for kt in range(NT):
                attn_e_kt(3, kt)
                attn_av_kt(2, kt)
            attn_fin(2)
            av_alloc(3)
            # av3 half 0 first so fin(3) half 0 can overlap av3 half 1
            for kt in range(NT):
                attn_av_kt(3, kt, (0,))
                yprojp(kt, (0, 1), y01, "act")
            attn_fin(3, (0,))
            for kt in range(NT):
                attn_av_kt(3, kt, (1,))
                if kt % 2 == 1:
                    yprojp(kt // 2, (2, 3), y23, "act")
            attn_fin(3, (1,))
            for nt in range(NT // 2, NT):
                yprojp(nt, (2, 3), y23, "act")

    nc.compile()
    return nc


def _get_nc():
    if "nc" not in _CACHE:
        _CACHE["nc"] = _build()
    return _CACHE["nc"]


def _bf16(a):
    return np.ascontiguousarray(a).astype(ml_dtypes.bfloat16)


def kernel(x, mask, Wq, bq, Wk, bk, Wv, bv, Wo, bo):
    x = np.asarray(x, dtype=np.float32)
    mask = np.asarray(mask)
    Wq = np.asarray(Wq, dtype=np.float32)
    Wk = np.asarray(Wk, dtype=np.float32)
    Wv = np.asarray(Wv, dtype=np.float32)
    Wo = np.asarray(Wo, dtype=np.float32)
    bq = np.asarray(bq, dtype=np.float32)
    bk = np.asarray(bk, dtype=np.float32)
    bv = np.asarray(bv, dtype=np.float32)
    bo = np.asarray(bo, dtype=np.float32)

    nc = _get_nc()

    in_maps = []
    for c in range(NCORES):
        b = c // 2
        hh = c % 2
        dsl = slice(hh * DPC, (hh + 1) * DPC)
        mbias = np.where(mask[b], MASK_NEG, 0.0).astype(np.float32)
        in_maps.append({
            "xT": _bf16(x[b].T),
            "wq": _bf16(Wq[dsl, :].T),
            "wk": _bf16(Wk[dsl, :].T),
            "wv": _bf16(Wv[dsl, :].T),
            "wo": _bf16(Wo[:, dsl].T),
            "bq": np.ascontiguousarray(bq[dsl].reshape(DT, 128).T),
            "bk": np.ascontiguousarray(bk[dsl].reshape(DT, 128).T),
            "bv": _bf16(bv[dsl].reshape(1, DPC)),
            "mb": np.ascontiguousarray(mbias.reshape(NT, 128).T),
        })

    res = None
    for attempt in range(3):
        try:
            res = run_bass_kernel_spmd(nc, in_maps,
                                       core_ids=list(range(NCORES)))
            break
        except Exception:
            # transient NRT/axon failures (e.g. NRT_EXEC_UNIT_UNRECOVERABLE)
            # recover on retry
            if attempt == 2:
                raise
            time.sleep(2.0)

    out = np.empty((B, N, D), dtype=np.float32)
    for b in range(B):
        r0 = res.results[2 * b]
        r1 = res.results[2 * b + 1]
        out[b] = ((r0["y01_part"] + r0["y23_part"])
                  + (r1["y01_part"] + r1["y23_part"]) + bo)
    return out



# revision 9
# speedup vs baseline: 1.0836x; 1.0836x over previous
"""Multi-head self-attention (B=4, N=1024, D=1024, H=16) on 8 Trainium2 NeuronCores.

Sharding: core c handles batch b = c//2 and head-half hh = c%2 (8 of 16 heads).
Each core computes Q/K/V projections for its (batch, head-half), the full
attention for its 8 heads, and a partial output projection over its 512
head-dims.  The host sums the two partial outputs per batch (pairwise
reduce, bf16 partials upcast to f32) and adds the output bias.

Device algorithm (all matmuls bf16 inputs, f32 PSUM accumulation):
  QT[dh, n]  = sum_e WqT[e, dh] * xT[e, n]      (+ bq per-partition bias add)
  KT[dh, n]  likewise
  V[n, dh]   = sum_e xT[e, n] * WvT[e, dh]      (+ bv via DVE add of a
                                                 host-replicated bias tile)
  eT[k, q]   = sum_d KT[d, k] * QT[d, q]        per (head, q-half) quarter
  PT[k, q]   = exp(eT * DK^-0.5 + maskbias[k])  (mask -> -30000 -> exp==0)
  attnT'[m,q]= sum_k V'[k, m] * PT[k, q]        V' has a ones column -> row 64
                                                 of attnT' is the softmax sum s
  rs         = 1/s (DVE reciprocal on the [1, n] PSUM row), broadcast to all
               128 partitions by ONE PE matmul with a [2,128] 0/1 weight
  at[dh, n]  = attnT'[dh, n] * rs[n]            (DVE)
  y[n, e]    = sum_dh at[dh, n] * WoT[dh, e]    partial over this core's dh

Schedule: energies are computed in [128,512] PSUM quarter-tiles so the
ScalarE exp stream drains them at fine granularity, and the attention
P@V' runs in half-query passes ([65,512] PSUM accumulators).  The Q/K
projections for later head pairs are woven into the exp-bound loops as
PE filler so the in-order PE queue never waits on ScalarE.  PSUM budget
(8 banks): acc tag 2x[128,1024] (projections + y), e tag 2x[128,512]
(energy quarters + 1/s broadcast), att tag 2x[65,512] (P@V' + V proj).
"""
import os
import sys
import time

for _p in (
    "/opt/trn_rl_repo",
    "/root/.axon_site",
    "/root/.axon_site/_ro/trn_rl_repo",
    "/root/.axon_site/_ro/pypackages",
):
    if os.path.isdir(_p) and _p not in sys.path:
        sys.path.append(_p)

import numpy as np
import ml_dtypes

import concourse.bacc as bacc
import concourse.tile as tile
from concourse import mybir
from concourse.bass_utils import run_bass_kernel_spmd

B, N, D, H = 4, 1024, 1024, 16
DK = D // H          # 64
NCORES = 8
HPC = H // 2         # 8 heads per core
DPC = D // 2         # 512 head-dims per core
NT = N // 128        # 8 token tiles
ET = D // 128        # 8 model-dim tiles
DT = DPC // 128      # 4 head-dim tiles (one per head pair)
SCALE = float(DK) ** -0.5
MASK_NEG = -30000.0
F32 = mybir.dt.float32
BF16 = mybir.dt.bfloat16

_CACHE = {}


def _build():
    nc = bacc.Bacc("TRN2", target_bir_lowering=False, debug=False,
                   num_devices=NCORES)
    xT = nc.dram_tensor("xT", [D, N], BF16, kind="ExternalInput")
    wq = nc.dram_tensor("wq", [D, DPC], BF16, kind="ExternalInput")
    wk = nc.dram_tensor("wk", [D, DPC], BF16, kind="ExternalInput")
    wv = nc.dram_tensor("wv", [D, DPC], BF16, kind="ExternalInput")
    wo = nc.dram_tensor("wo", [DPC, D], BF16, kind="ExternalInput")
    bq = nc.dram_tensor("bq", [128, DT], F32, kind="ExternalInput")
    bk = nc.dram_tensor("bk", [128, DT], F32, kind="ExternalInput")
    bvr = nc.dram_tensor("bvr", [128, DPC], BF16, kind="ExternalInput")
    mb = nc.dram_tensor("mb", [128, NT], F32, kind="ExternalInput")
    w2c = nc.dram_tensor("w2c", [2, 128], BF16, kind="ExternalInput")
    y01 = nc.dram_tensor("y01_part", [N, D], BF16, kind="ExternalOutput")
    y23 = nc.dram_tensor("y23_part", [N, D], BF16, kind="ExternalOutput")

    with tile.TileContext(nc) as tc:
        with tc.tile_pool(name="sb", bufs=1) as sb, \
             tc.tile_pool(name="work", bufs=2) as wp, \
             tc.tile_pool(name="ps", bufs=2, space="PSUM") as ps:

            # ---------------- persistent SBUF + input loads ----------------
            # queue A (sync): wq + xT(1..7), needed first for the Q proj.
            # queue B (gpsimd): everything else.
            xT_sb = sb.tile([128, ET, N], BF16)
            wq_sb = sb.tile([128, ET, DPC], BF16)
            wk_sb = sb.tile([128, ET, DPC], BF16)
            wv_sb = sb.tile([128, ET, DPC], BF16)
            wo_sb = sb.tile([128, DT, D], BF16)
            bq_sb = sb.tile([128, DT], F32)
            bk_sb = sb.tile([128, DT], F32)
            mb_sb = sb.tile([128, NT], F32)
            bvr_sb = sb.tile([128, DPC], BF16)

            nc.gpsimd.dma_start(out=xT_sb[:, 0, :], in_=xT.ap()[0:128, :])
            nc.gpsimd.dma_start(out=bq_sb, in_=bq.ap())
            nc.gpsimd.dma_start(out=bk_sb, in_=bk.ap())
            nc.gpsimd.dma_start(out=mb_sb, in_=mb.ap())
            for et in range(0, ET):
                nc.sync.dma_start(out=wq_sb[:, et, :],
                                  in_=wq.ap()[et * 128:(et + 1) * 128, :])
                if et > 0:
                    nc.sync.dma_start(out=xT_sb[:, et, :],
                                      in_=xT.ap()[et * 128:(et + 1) * 128, :])
                nc.gpsimd.dma_start(out=wk_sb[:, et, :],
                                    in_=wk.ap()[et * 128:(et + 1) * 128, :])
            nc.gpsimd.dma_start(out=bvr_sb, in_=bvr.ap())
            for et in range(0, ET):
                nc.gpsimd.dma_start(out=wv_sb[:, et, :],
                                    in_=wv.ap()[et * 128:(et + 1) * 128, :])
            for dt in range(DT):
                nc.gpsimd.dma_start(out=wo_sb[:, dt, :],
                                    in_=wo.ap()[dt * 128:(dt + 1) * 128, :])

            qt_sb = sb.tile([128, DT, N], BF16)
            kt_sb = sb.tile([128, DT, N], BF16)
            v_sb = sb.tile([128, NT, HPC, DK + 1], BF16)
            at_sb = sb.tile([128, DT, N], BF16)
            # 1/s rows for the A heads (partition 0) and B heads (partition
            # 64) — DVE can only write at partition starts 0/32/64/96; the
            # broadcast matmul reads both rows via a stride-64 partition AP
            rs2 = sb.tile([65, N], BF16)
            # 0/1 weight that broadcasts rs2 row 0 -> partitions 0-63 and
            # row 1 -> partitions 64-127 in a single PE pass (host constant)
            w2 = sb.tile([2, 128], BF16)
            nc.gpsimd.dma_start(out=w2, in_=w2c.ap())
            # ones column of V' (row DK of each head's V block)
            nc.vector.memset(v_sb[:, :, :, DK:DK + 1], 1.0)

            # ---------------- unit generators (PE-queue weaving) ----------
            # Each yields after emitting ~1-2 matmuls so the driver can
            # interleave streams; drains (DVE/ACT) are emitted inline.

            def pq_full(m, dt):
                # Q/K projection for one dt (128 head dims), full N.
                w_sb, b_sb, dst = ((wq_sb, bq_sb, qt_sb),
                                   (wk_sb, bk_sb, kt_sb))[m]
                pq = ps.tile([128, N], F32, tag="acc", name=f"pq{m}_{dt}")
                for et in range(ET):
                    for h in range(2):
                        qs = slice(h * 512, (h + 1) * 512)
                        nc.tensor.matmul(pq[:, qs],
                                         w_sb[:, et, dt * 128:(dt + 1) * 128],
                                         xT_sb[:, et, qs],
                                         start=(et == 0), stop=(et == ET - 1))
                    yield 2
                nc.vector.tensor_scalar_add(dst[:, dt, :], pq,
                                            b_sb[:, dt:dt + 1])

            def pq_half(m, dt, h):
                # same, half-N, in a 1-bank "e" slot (startup filler)
                w_sb, b_sb, dst = ((wq_sb, bq_sb, qt_sb),
                                   (wk_sb, bk_sb, kt_sb))[m]
                qs = slice(h * 512, (h + 1) * 512)
                pq = ps.tile([128, 512], F32, tag="e", name=f"pqh{m}_{dt}_{h}")
                for et in range(ET):
                    nc.tensor.matmul(pq,
                                     w_sb[:, et, dt * 128:(dt + 1) * 128],
                                     xT_sb[:, et, qs],
                                     start=(et == 0), stop=(et == ET - 1))
                    yield 1
                nc.vector.tensor_scalar_add(dst[:, dt, qs], pq,
                                            b_sb[:, dt:dt + 1])

            def pv_unit(nt):
                pv = ps.tile([128, 512], F32, tag="acc", name=f"pv{nt}")
                ns = slice(nt * 128, (nt + 1) * 128)
                for et in range(ET):
                    nc.tensor.matmul(pv, xT_sb[:, et, ns], wv_sb[:, et, :],
                                     start=(et == 0), stop=(et == ET - 1))
                    yield 1
                nc.vector.tensor_tensor(
                    out=v_sb[:, nt, :, 0:DK],
                    in0=pv.rearrange("p (h d) -> p h d", h=HPC),
                    in1=bvr_sb.rearrange("p (h d) -> p h d", h=HPC),
                    op=mybir.AluOpType.add)

            pt = {}

            def eq_gen(p):
                # energies + exp for head pair p, quarter granularity
                ptA = wp.tile([128, NT, N], BF16, tag="pt", bufs=4,
                              name=f"ptA{p}")
                ptB = wp.tile([128, NT, N], BF16, tag="pt", bufs=4,
                              name=f"ptB{p}")
                pt[p] = (ptA, ptB)
                for kt in range(NT):
                    ks = slice(kt * 128, (kt + 1) * 128)
                    for ab, h in ((0, 0), (1, 0), (0, 1), (1, 1)):
                        qs = slice(h * 512, (h + 1) * 512)
                        rows = slice(64 * ab, 64 * (ab + 1))
                        e = ps.tile([128, 512], F32, tag="e",
                                    name=f"e{p}_{kt}_{ab}{h}")
                        nc.tensor.matmul(e, kt_sb[rows, p, ks],
                                         qt_sb[rows, p, qs],
                                         start=True, stop=True)
                        nc.scalar.activation((ptA, ptB)[ab][:, kt, qs], e,
                                             mybir.ActivationFunctionType.Exp,
                                             bias=mb_sb[:, kt:kt + 1],
                                             scale=SCALE)
                        yield 1

            av = {}

            def av_gen(p, h):
                # P@V' accumulation for one q-half of head pair p
                qs = slice(h * 512, (h + 1) * 512)
                aA = ps.tile([65, 512], F32, tag="att", name=f"aA{p}_{h}")
                aB = ps.tile([65, 512], F32, tag="att", name=f"aB{p}_{h}")
                av[(p, h)] = (aA, aB)
                ptA, ptB = pt[p]
                for kt in range(NT):
                    nc.tensor.matmul(aA, v_sb[:, kt, 2 * p, :],
                                     ptA[:, kt, qs],
                                     start=(kt == 0), stop=(kt == NT - 1))
                    nc.tensor.matmul(aB, v_sb[:, kt, 2 * p + 1, :],
                                     ptB[:, kt, qs],
                                     start=(kt == 0), stop=(kt == NT - 1))
                    yield 2

            def fin(p, h, chunks=1):
                # softmax normalization for one q-half of pair p
                aA, aB = av[(p, h)]
                cw = 512 // chunks
                for c in range(chunks):
                    lo = c * cw
                    g = slice(h * 512 + lo, h * 512 + lo + cw)
                    loc = slice(lo, lo + cw)
                    with nc.allow_low_precision(reason="softmax 1/s in bf16"):
                        nc.vector.reciprocal(rs2[0:1, g], aA[64:65, loc])
                        nc.vector.reciprocal(rs2[64:65, g], aB[64:65, loc])
                    srep = ps.tile([128, cw], F32, tag="e",
                                   name=f"srep{p}_{h}_{c}",
                                   padded_shape=[128, 512])
                    nc.tensor.matmul(srep, w2, rs2[0:65:64, g],
                                     start=True, stop=True)
                    nc.vector.tensor_tensor(out=at_sb[0:64, p, g],
                                            in0=aA[0:64, loc],
                                            in1=srep[0:64, :],
                                            op=mybir.AluOpType.mult)
                    nc.vector.tensor_tensor(out=at_sb[64:128, p, g],
                                            in0=aB[0:64, loc],
                                            in1=srep[64:128, :],
                                            op=mybir.AluOpType.mult)

            def y_unit(nt, dts, ydram, copy_eng):
                yp = ps.tile([128, N], F32, tag="acc",
                             name=f"yp{dts[0]}_{nt}")
                ns = slice(nt * 128, (nt + 1) * 128)
                for h in range(2):
                    qs = slice(h * 512, (h + 1) * 512)
                    for dt in dts:
                        nc.tensor.matmul(yp[:, qs], at_sb[:, dt, ns],
                                         wo_sb[:, dt, qs],
                                         start=(dt == dts[0]),
                                         stop=(dt == dts[-1]))
                    yield 2
                ysb = wp.tile([128, N], BF16, tag="y", bufs=4,
                              name=f"ysb{dts[0]}_{nt}")
                if copy_eng == "act":
                    nc.scalar.copy(ysb, yp)
                else:
                    nc.vector.tensor_copy(out=ysb, in_=yp)
                nc.gpsimd.dma_start(out=ydram.ap()[ns, :], in_=ysb)

            def run(gen):
                for _ in gen:
                    pass

            def step(gen, n=1):
                # pull up to n yields; True while gen still live
                for _ in range(n):
                    try:
                        next(gen)
                    except StopIteration:
                        return False
                return True

            def chain(*gens):
                for g in gens:
                    yield from g

            # ------------- emission order (software pipeline) -------------
            # S: Q/K dt0 (acc slots) + Q dt1 halves (e slots), DMA-paced.
            # 6 matmuls per et step, tracking the input-DMA arrival rate.
            s_units = [pq_full(0, 0), pq_full(1, 0),
                       pq_half(0, 1, 0), pq_half(0, 1, 1)]
            for et in range(ET):
                for g in s_units:
                    step(g)
            # P0: K dt1 (acc, early), energies p0, V projections.  pv units
            # rotate through the acc bufs behind Kd1; pv1 is held until Kd1
            # is past so its buf-wait never starves the in-order PE queue.
            kd1 = pq_full(1, 1)
            eq0 = eq_gen(0)
            pvc = chain(*[pv_unit(nt) for nt in range(NT)])
            for s in range(32):
                step(eq0)
                step(kd1)                      # 2 mm, slots 0-7
                if s < 4:
                    step(pvc, 2)               # pv0; pv1 held past Kd1
                elif s >= 8:
                    step(pvc, 3 if s % 3 == 2 else 2)  # 56 mm / 24 slots
            while step(pvc, 2):
                pass
            run(eq0)
            run(kd1)

            def attn_phase(p, pq_m_dts, y_nts, y_dram):
                # energies p+1 woven with av(p) passes, plus Q/K projection
                # or y-projection filler.  ~36 slots, ~3 mm each; av pass
                # starts are padded with eq+filler slots so PSUM buf-rotation
                # waits (prev pass's fin chain) never starve the PE queue.
                eq = eq_gen(p + 1)
                avh0 = av_gen(p, 0)
                pqs = [pq_full(m, dt) for m, dt in pq_m_dts]
                yus = [y_unit(nt, (0, 1), y_dram, "dve") for nt in y_nts]
                fillers = chain(*pqs, *yus)
                for s in range(4):             # cover prev fin chain
                    step(eq)
                    step(fillers)
                for s in range(16):
                    step(eq)
                    step(avh0)
                    if s % 2 == 0:
                        step(fillers)
                fin(p, 0)
                for s in range(4):             # cover fin(p,0) chain
                    step(eq)
                    step(fillers)
                avh1 = av_gen(p, 1)
                for s in range(12):
                    step(eq)
                    step(avh1, 2 if s % 3 == 2 else 1)
                    if s % 2 == 0:
                        step(fillers)
                while step(avh1):
                    step(eq)
                    step(fillers)
                while step(eq):
                    step(fillers)
                while step(fillers):
                    pass
                fin(p, 1)

            # P1: energies p1 + av0 + Q/K dt2
            attn_phase(0, [(0, 2), (1, 2)], [], None)
            # P2: energies p2 + av1 + Q/K dt3
            attn_phase(1, [(0, 3), (1, 3)], [], None)
            # P3: energies p3 + av2 + y01
            attn_phase(2, [], range(NT), y01)
            # P4 tail: av3 h0, then av3 h1 overlapped with y23 nt0-3,
            # chunked fin(3,h1), y23 nt4-7
            av30 = av_gen(3, 0)
            run(av30)
            fin(3, 0)
            av31 = av_gen(3, 1)
            y23a = chain(*[y_unit(nt, (2, 3), y23, "act")
                           for nt in range(0, 4)])
            while step(av31):
                step(y23a)
            while step(y23a):
                pass
            fin(3, 1, chunks=4)
            for nt in range(4, NT):
                run(y_unit(nt, (2, 3), y23, "act"))

    nc.compile()
    return nc


def _get_nc():
    if "nc" not in _CACHE:
        _CACHE["nc"] = _build()
    return _CACHE["nc"]


def _bf16(a):
    return np.ascontiguousarray(a).astype(ml_dtypes.bfloat16)


def kernel(x, mask, Wq, bq, Wk, bk, Wv, bv, Wo, bo):
    x = np.asarray(x, dtype=np.float32)
    mask = np.asarray(mask)
    Wq = np.asarray(Wq, dtype=np.float32)
    Wk = np.asarray(Wk, dtype=np.float32)
    Wv = np.asarray(Wv, dtype=np.float32)
    Wo = np.asarray(Wo, dtype=np.float32)
    bq = np.asarray(bq, dtype=np.float32)
    bk = np.asarray(bk, dtype=np.float32)
    bv = np.asarray(bv, dtype=np.float32)
    bo = np.asarray(bo, dtype=np.float32)

    nc = _get_nc()

    in_maps = []
    for c in range(NCORES):
        b = c // 2
        hh = c % 2
        dsl = slice(hh * DPC, (hh + 1) * DPC)
        mbias = np.where(mask[b], MASK_NEG, 0.0).astype(np.float32)
        in_maps.append({
            "xT": _bf16(x[b].T),
            "wq": _bf16(Wq[dsl, :].T),
            "wk": _bf16(Wk[dsl, :].T),
            "wv": _bf16(Wv[dsl, :].T),
            "wo": _bf16(Wo[:, dsl].T),
            "bq": np.ascontiguousarray(bq[dsl].reshape(DT, 128).T),
            "bk": np.ascontiguousarray(bk[dsl].reshape(DT, 128).T),
            "bvr": _bf16(np.broadcast_to(bv[dsl][None, :], (128, DPC))),
            "mb": np.ascontiguousarray(mbias.reshape(NT, 128).T),
            "w2c": _bf16(np.kron(np.eye(2, dtype=np.float32),
                                 np.ones((1, 64), dtype=np.float32))),
        })

    res = None
    for attempt in range(3):
        try:
            res = run_bass_kernel_spmd(nc, in_maps,
                                       core_ids=list(range(NCORES)))
            break
        except Exception:
            # transient NRT/axon failures (e.g. NRT_EXEC_UNIT_UNRECOVERABLE)
            # recover on retry
            if attempt == 2:
                raise
            time.sleep(2.0)

    out = np.empty((B, N, D), dtype=np.float32)
    for b in range(B):
        r0 = res.results[2 * b]
        r1 = res.results[2 * b + 1]
        out[b] = ((r0["y01_part"].astype(np.float32)
                   + r0["y23_part"].astype(np.float32))
                  + (r1["y01_part"].astype(np.float32)
                     + r1["y23_part"].astype(np.float32)) + bo)
    return out
